# revision 13
# baseline (speedup 1.0000x reference)
"""Trainium2 Bass kernel for the DispaxD3 two-body dispersion energy.

Strategy (8 NeuronCores, SPMD):
  - Edges are sorted by their i-atom and sharded across cores at atom
    boundaries.  Per-core edge slots are laid out in degree-bucketed padded
    runs [128 partitions, n_atom_cols, L] so the per-atom segment sum and
    broadcasts are regular strided vector ops.
  - Launch 1 (full 3.2M-edge stream, fp16): computes coordination numbers,
    then the normalized gaussian reference weights AND the raw weight sum
    per atom, all on device.
  - Pruning: the gaussian weight sum w_a = sum_r exp(-4 (ref_ar - cn_a)^2)
    underflows for all but ~8% of atoms (cn is large); an edge's C6 term is
    bounded by ~2.7e-3 * min(1, w_i/eps) * min(1, w_j/eps), so edges where
    either endpoint has w < 1e-12 contribute < 2e-6 relative in total.  The
    host keeps only edges with BOTH endpoints active (pure comparisons and
    gathers, no host arithmetic) -- ~0.7% of edges survive.
  - Launch 2 runs the exact bf16 5x5 C6 bilinear + BJ damping on the tiny
    kept-edge stream, segment-reduces per atom, dots with the i-atom
    weights and reduces to one scalar per core.  Host sums the 8 partials
    (the "all-reduce").
  - All element data is host-gathered into the streams; all floating point
    math happens on device.  Host does joins/selection/layout only.
"""

import sys

sys.path.insert(0, "/opt/trn_rl_repo")

from contextlib import ExitStack

import ml_dtypes
import numpy as np

import concourse.bacc as bacc
import concourse.bass as bass
import concourse.mybir as mybir
import concourse.tile as tile
from concourse.bass_utils import run_bass_kernel_spmd

F32 = mybir.dt.float32
F16 = mybir.dt.float16
BF16 = mybir.dt.bfloat16
AF = mybir.ActivationFunctionType
ALU = mybir.AluOpType
AX = mybir.AxisListType

BOHR = 0.5291772105638411
HA = 27.211386024367243
S6, S8, A1, A2 = 1.0, 0.7875, 0.4289, 4.4407
KCN = 16.0
WF = 4.0
EPS32 = float(np.finfo(np.float32).eps)
WSUM_CUT = 1e-12  # activity threshold on the raw gaussian weight sum

NCORES = 8
P = 128
LS1 = [12, 16, 20, 24, 28, 32, 36, 40, 44, 48, 56, 64, 96, 128, 256, 384]
LS2 = [16, 32, 64, 128, 256, 384]
MAXCOLS1 = 1536
MAXCOLS2 = 576
RCJ_PAD = -4.0  # pad-slot rcov_j: makes rc<=-2.5 so sigmoid count underflows to 0

SLOT1 = 4  # launch-1 planar fp16 lanes: dx dy dz rcov_j
NL2 = 34  # launch-2 planar bf16 lanes: dx dy dz r4r2_j wj[5] c6block[25]

_cache = {}
REPEAT = 1
TRACE = False
LAST_R1 = None
LAST_R2 = None


def _geometry(deg_lists, LS, maxcols):
    """Unified piece geometry from per-core degree lists (atoms to place)."""
    percore = []
    for degs in deg_lists:
        li = np.searchsorted(LS, degs, side="left")
        if len(degs):
            assert li.max() < len(LS), f"degree {degs.max()} exceeds bucket table"
        percore.append(np.bincount(li, minlength=len(LS)))
    nmax = np.stack(percore).max(axis=0)
    nmax = ((nmax + P - 1) // P) * P

    pieces = []  # (L, n_p, scol_off, acol_off)
    groups = []  # per bucket: (L, n_atoms, scol_off, acol_off)
    scol = 0
    acol = 0
    for bi, L in enumerate(LS):
        n = int(nmax[bi])
        groups.append((L, n, scol, acol))
        if n == 0:
            continue
        n_cols = n // P
        npp = max(1, maxcols // L)
        c = 0
        while c < n_cols:
            take = min(npp, n_cols - c)
            pieces.append((L, take, scol + c * L, acol + c))
            c += take
        scol += n_cols * L
        acol += n_cols
    return pieces, groups, scol, acol


def _place(li, atom_ids, groups, ACOLS):
    """Per-core placement of atoms (ascending ids) into the bucket grid."""
    n = len(atom_ids)
    part = np.empty(n, np.int64)
    scolb = np.empty(n, np.int64)
    agrid = np.full((P, ACOLS), -1, np.int64)
    for bi, (L, _, scol0, acol0) in enumerate(groups):
        sel = np.nonzero(li == bi)[0]
        if len(sel) == 0:
            continue
        t = np.arange(len(sel))
        c = t // P
        p = t % P
        part[sel] = p
        scolb[sel] = scol0 + c * L
        agrid[p, acol0 + c] = atom_ids[sel]
    return part, scolb, agrid


def _prep_static(dr_vec, ref_cn_table, ref_c6_table, r4r2_table, rcov_table, numbers, idx):
    N = numbers.shape[0]
    E = idx.shape[1]
    i = idx[0].astype(np.int64)
    j = idx[1].astype(np.int64)

    counts = np.bincount(i, minlength=N)
    ccum = np.concatenate([[0], np.cumsum(counts)])
    targets = [E * k // NCORES for k in range(1, NCORES)]
    cuts = [0] + [int(np.searchsorted(ccum, t)) for t in targets] + [N]
    atom_ranges = [(cuts[k], cuts[k + 1]) for k in range(NCORES)]

    deg1 = [counts[a0:a1] for a0, a1 in atom_ranges]
    pieces, groups, COLS, ACOLS = _geometry(deg1, LS1, MAXCOLS1)

    order = np.argsort(i, kind="stable")
    i_s = i[order]
    pos = np.arange(E, dtype=np.int64) - ccum[i_s]

    rcov_a = rcov_table[numbers]
    refcn_a = ref_cn_table[numbers]

    cores = []
    for k, (a0, a1) in enumerate(atom_ranges):
        degs = counts[a0:a1]
        li = np.searchsorted(LS1, degs, side="left")
        part, scolb, agrid = _place(li, np.arange(a0, a1), groups, ACOLS)

        e0, e1 = ccum[a0], ccum[a1]
        eo = order[e0:e1]
        il = i_s[e0:e1] - a0
        pp = part[il]
        cc = scolb[il] + pos[e0:e1]

        s1 = np.zeros((P, SLOT1, COLS), np.float16)
        s1[:, 3, :] = RCJ_PAD
        s1[pp, 0, cc] = dr_vec[eo, 0]
        s1[pp, 1, cc] = dr_vec[eo, 1]
        s1[pp, 2, cc] = dr_vec[eo, 2]
        s1[pp, 3, cc] = rcov_a[j[eo]]

        am = agrid >= 0
        at1 = np.zeros((P, 6, ACOLS), np.float32)
        at1[:, 0, :][am] = rcov_a[agrid[am]]
        tmp = np.zeros((P, ACOLS, 5), np.float32)
        tmp[am] = refcn_a[agrid[am]]
        at1[:, 1:6, :] = tmp.transpose(0, 2, 1)

        cores.append(dict(s1=s1, at1=at1, agrid=agrid, am=am, eo=eo, a0=a0, a1=a1))

    return dict(
        pieces=pieces, COLS=COLS, ACOLS=ACOLS, cores=cores, N=N, E=E,
        i=i, j=j, dr_vec=dr_vec, numbers=numbers,
        r4r2_table=r4r2_table, ref_c6_table=ref_c6_table,
        atom_ranges=atom_ranges,
    )


def _join(prep, w5_list, wsum_list):
    """Select contributing edges from launch-1's per-atom weight sums and
    build the compact launch-2 inputs.  Comparisons + gathers only."""
    N, ACOLS = prep["N"], prep["ACOLS"]
    i, j = prep["i"], prep["j"]
    numbers = prep["numbers"]
    dr_vec = prep["dr_vec"]
    r4r2_a = prep["r4r2_table"][numbers]

    ws_full = np.zeros(N, np.float32)
    w_full = np.zeros((N, 5), np.float32)
    grids = []
    for k, c in enumerate(prep["cores"]):
        m = c["am"]
        w5r = np.asarray(w5_list[k]).reshape(P, 5, ACOLS).transpose(0, 2, 1)
        grids.append(w5r)
        ws_full[c["agrid"][m]] = np.asarray(wsum_list[k]).reshape(P, ACOLS)[m]
        w_full[c["agrid"][m]] = w5r[m]

    active = ws_full > WSUM_CUT
    keep = active[i] & active[j]
    kcounts = np.bincount(i[keep], minlength=N)

    deg2 = []
    aid2 = []
    for a0, a1 in prep["atom_ranges"]:
        loc = kcounts[a0:a1]
        sel = np.nonzero(loc > 0)[0]
        deg2.append(loc[sel])
        aid2.append(sel + a0)

    pieces2, groups2, COLS2, ACOLS2 = _geometry(deg2, LS2, MAXCOLS2)

    r25 = np.arange(25)
    r5 = np.arange(5)
    cores2 = []
    for k, c in enumerate(prep["cores"]):
        li2 = np.searchsorted(LS2, deg2[k], side="left")
        part2, scolb2, agrid2 = _place(li2, aid2[k], groups2, ACOLS2)

        eo = c["eo"]
        ke = eo[keep[eo]]  # kept edges, still i-sorted
        ki = i[ke]
        fidx = np.searchsorted(aid2[k], ki)  # position of atom in aid2
        kpos = np.arange(len(ki)) - np.searchsorted(ki, ki)  # rank within run
        pp2 = part2[fidx]
        cc2 = scolb2[fidx] + kpos

        kj = j[ke]
        s2 = np.zeros((P, NL2, COLS2), ml_dtypes.bfloat16)
        s2[pp2, 0, cc2] = dr_vec[ke, 0]
        s2[pp2, 1, cc2] = dr_vec[ke, 1]
        s2[pp2, 2, cc2] = dr_vec[ke, 2]
        s2[pp2, 3, cc2] = r4r2_a[kj]
        s2[pp2[:, None], 4 + r5[None, :], cc2[:, None]] = w_full[kj]
        bfk = prep["ref_c6_table"][numbers[kj], numbers[ki]].reshape(-1, 25)
        s2[pp2[:, None], 9 + r25[None, :], cc2[:, None]] = bfk.astype(ml_dtypes.bfloat16)

        am2 = agrid2 >= 0
        at2 = np.zeros((P, 6, ACOLS2), np.float32)
        at2[:, 0, :][am2] = r4r2_a[agrid2[am2]]
        tmp = np.zeros((P, ACOLS2, 5), np.float32)
        tmp[am2] = w_full[agrid2[am2]]
        at2[:, 1:6, :] = tmp.transpose(0, 2, 1)

        cores2.append(dict(s2=s2, at2=at2))

    return dict(pieces=pieces2, COLS=COLS2, ACOLS=ACOLS2, cores=cores2)


def _new_nc():
    return bacc.Bacc("TRN2", target_bir_lowering=False, debug=False, num_devices=NCORES)


def _build_l1(pieces, COLS, ACOLS):
    nc = _new_nc()
    s1 = nc.declare_dram_parameter("s1", [P, SLOT1 * COLS], F16, isOutput=False)
    at1 = nc.declare_dram_parameter("at1", [P, 6 * ACOLS], F32, isOutput=False)
    w5o = nc.declare_dram_parameter("w5", [P, 5 * ACOLS], F32, isOutput=True)
    wso = nc.declare_dram_parameter("wsum", [P, ACOLS], F32, isOutput=True)
    s1v = s1[:].rearrange("p (m c) -> p m c", m=SLOT1)

    with ExitStack() as ctx:
        tc = ctx.enter_context(tile.TileContext(nc))
        persist = ctx.enter_context(tc.tile_pool(name="persist", bufs=1))
        spool = ctx.enter_context(tc.tile_pool(name="stream", bufs=3))
        wpool = ctx.enter_context(tc.tile_pool(name="work", bufs=3))

        cn_t = persist.tile([P, ACOLS], F32)
        at_t = persist.tile([P, 6, ACOLS], F32)
        nc.sync.dma_start(at_t[:], at1[:].rearrange("p (f a) -> p f a", f=6))
        b_tiny = persist.tile([P, 1], F32)
        # dr^2 floor 1e-4 Bohr^2: keeps rdr <= 100 so fp16 targ cannot
        # overflow; distorts only edges with dr < 0.01 Bohr (none expected)
        nc.vector.memset(b_tiny[:], 1e-4)
        b_negk = persist.tile([P, 1], F32)
        nc.vector.memset(b_negk[:], -KCN)
        targ_f = persist.tile([P, COLS], F16)
        cnt_f = persist.tile([P, COLS], F16)

        # Main loop keeps the ACT engine inside one activation-table set
        # (square+sqrt coexist); sigmoid runs as a second pass so the 1.3us
        # table load happens once per launch instead of twice per piece.
        for _rep in range(REPEAT):
          for (L, n_p, scol, acol) in pieces:
            W = n_p * L
            st = spool.tile([P, SLOT1, W], F16, tag="st")
            nc.sync.dma_start(st[:], s1v[:, :, scol:scol + W])

            sq3 = wpool.tile([P, 3, W], F16, tag="sq3")
            nc.scalar.activation(sq3[:], st[:, 0:3, :], AF.Square)
            s_ = wpool.tile([P, W], F16, tag="s_")
            # |d|^2 adds on GPSIMD: frees DVE, which is the L1 bottleneck
            nc.gpsimd.tensor_tensor(s_[:], sq3[:, 0, :], sq3[:, 1, :], ALU.add)
            nc.gpsimd.tensor_tensor(s_[:], s_[:], sq3[:, 2, :], ALU.add)
            dr = wpool.tile([P, W], F32, tag="dr")
            # dr = sqrt(|d|^2/BOHR^2 + tiny); tiny keeps pad slots finite
            nc.scalar.activation(dr[:], s_[:], AF.Sqrt, scale=1.0 / BOHR**2, bias=b_tiny[:])
            rdr = wpool.tile([P, W], F32, tag="rdr")
            nc.vector.reciprocal_approx_fast(rdr[:], dr[:])
            rc = wpool.tile([P, W], F16, tag="rc")
            rci = at_t[:, 0, acol:acol + n_p].unsqueeze(-1).to_broadcast([P, n_p, L])
            nc.vector.tensor_tensor(
                rc[:].rearrange("p (a l) -> p a l", a=n_p),
                st[:, 3, :].rearrange("p (a l) -> p a l", a=n_p), rci, ALU.add,
            )
            # pad slots: rc <= -2.5, rdr ~ 1e15 -> targ very negative -> count 0
            nc.vector.tensor_tensor(targ_f[:, scol:scol + W], rc[:], rdr[:], ALU.mult)

          for (L, n_p, scol, acol) in pieces:
            W = n_p * L
            nc.scalar.activation(
                cnt_f[:, scol:scol + W], targ_f[:, scol:scol + W],
                AF.Sigmoid, scale=KCN, bias=b_negk[:],
            )
            nc.vector.tensor_reduce(
                cn_t[:, acol:acol + n_p],
                cnt_f[:, scol:scol + W].rearrange("p (a l) -> p a l", a=n_p),
                AX.X, ALU.add,
            )

        # ---- per-atom gaussian weights from cn (tiny: 5*ACOLS elements)
        w5p = persist.tile([P, 5, ACOLS], F32)
        nc.vector.tensor_tensor(
            w5p[:], at_t[:, 1:6, :],
            cn_t[:].unsqueeze(1).to_broadcast([P, 5, ACOLS]), ALU.subtract,
        )
        nc.scalar.activation(w5p[:], w5p[:], AF.Square)
        nc.scalar.activation(w5p[:], w5p[:], AF.Exp, scale=-WF)
        wsum = persist.tile([P, ACOLS], F32)
        nc.vector.tensor_tensor(wsum[:], w5p[:, 0, :], w5p[:, 1, :], ALU.add)
        nc.vector.tensor_tensor(wsum[:], wsum[:], w5p[:, 2, :], ALU.add)
        nc.vector.tensor_tensor(wsum[:], wsum[:], w5p[:, 3, :], ALU.add)
        nc.vector.tensor_tensor(wsum[:], wsum[:], w5p[:, 4, :], ALU.add)
        nc.sync.dma_start(wso[:], wsum[:])
        wse = persist.tile([P, ACOLS], F32)
        nc.vector.tensor_scalar_add(wse[:], wsum[:], EPS32)
        winv = persist.tile([P, ACOLS], F32)
        nc.vector.reciprocal_approx_fast(winv[:], wse[:])
        nc.vector.tensor_tensor(
            w5p[:], w5p[:], winv[:].unsqueeze(1).to_broadcast([P, 5, ACOLS]), ALU.mult
        )
        nc.sync.dma_start(w5o[:].rearrange("p (f a) -> p f a", f=5), w5p[:])
    nc.compile()
    return nc


def _build_l2(pieces, COLS, ACOLS):
    nc = _new_nc()
    s2 = nc.declare_dram_parameter("s2", [P, NL2 * COLS], BF16, isOutput=False)
    at2 = nc.declare_dram_parameter("at2", [P, 6 * ACOLS], F32, isOutput=False)
    eto = nc.declare_dram_parameter("etot", [1, 1], F32, isOutput=True)
    s2v = s2[:].rearrange("p (m c) -> p m c", m=NL2)

    DSCALE = -0.5 * HA * S8  # energy scale folded into the damping factor

    with ExitStack() as ctx:
        tc = ctx.enter_context(tile.TileContext(nc))
        persist = ctx.enter_context(tc.tile_pool(name="persist", bufs=1))
        spool = ctx.enter_context(tc.tile_pool(name="stream", bufs=2))
        wpool = ctx.enter_context(tc.tile_pool(name="work", bufs=2))
        w5pool = ctx.enter_context(tc.tile_pool(name="work5", bufs=2))
        bpool = ctx.enter_context(tc.tile_pool(name="workb", bufs=2))
        ppool = ctx.enter_context(tc.tile_pool(name="psum", bufs=1, space="PSUM"))

        att = persist.tile([P, 6, ACOLS], F32)
        nc.sync.dma_start(att[:], at2[:].rearrange("p (f a) -> p f a", f=6))
        b_a2 = persist.tile([P, 1], F32)
        nc.vector.memset(b_a2[:], A2)
        r43 = persist.tile([P, ACOLS], F32)
        nc.vector.tensor_scalar_mul(r43[:], att[:, 0, :], 3.0)

        ecols = []
        for _rep in range(REPEAT):
          for pi, (L, n_p, scol, acol) in enumerate(pieces):
            W = n_p * L
            st = spool.tile([P, NL2, W], BF16, tag="st")
            nc.sync.dma_start(st[:], s2v[:, :, scol:scol + W])
            xyz = st[:, 0:3, :]
            r4 = st[:, 3, :]
            wj = st[:, 4:9, :]
            mb = st[:, 9:34, :]

            def wt(tag):
                return wpool.tile([P, W], F32, tag=tag, name=tag)

            def bt(tag):
                return bpool.tile([P, W], BF16, tag=tag, name=tag)

            sq3 = bpool.tile([P, 3, W], BF16, tag="sq3")
            nc.scalar.activation(sq3[:], xyz, AF.Square)
            s_ = bt("s_")
            nc.vector.tensor_tensor(s_[:], sq3[:, 0, :], sq3[:, 1, :], ALU.add)
            nc.vector.tensor_tensor(s_[:], s_[:], sq3[:, 2, :], ALU.add)
            t_ = bt("t_")
            nc.scalar.activation(t_[:], s_[:], AF.Square, scale=1.0 / BOHR**2)
            dr6 = bt("dr6")
            nc.vector.scalar_tensor_tensor(
                dr6[:], t_[:], 1.0 / BOHR**2, s_[:], ALU.mult, ALU.mult
            )
            dr8 = bt("dr8")
            nc.vector.scalar_tensor_tensor(
                dr8[:], dr6[:], 1.0 / BOHR**2, s_[:], ALU.mult, ALU.mult
            )
            qq = wt("qq")
            r4ib = r43[:, acol:acol + n_p].unsqueeze(-1).to_broadcast([P, n_p, L])
            nc.vector.tensor_tensor(
                qq[:].rearrange("p (a l) -> p a l", a=n_p),
                r4.rearrange("p (a l) -> p a l", a=n_p), r4ib, ALU.mult,
            )
            rrs = bt("rrs")
            nc.scalar.activation(rrs[:], qq[:], AF.Sqrt, scale=A1 * A1)
            rr2 = bt("rr2")
            nc.scalar.activation(rr2[:], rrs[:], AF.Square, bias=b_a2[:])
            t2_ = bt("rrs")
            nc.scalar.activation(t2_[:], rr2[:], AF.Square)
            rr6 = bt("rr6")
            nc.vector.tensor_tensor(rr6[:], t2_[:], rr2[:], ALU.mult)
            den6 = wt("den6")
            nc.vector.tensor_tensor(den6[:], dr6[:], rr6[:], ALU.add)
            i6 = wt("i6")
            nc.vector.reciprocal_approx_fast(i6[:], den6[:])
            nc.vector.tensor_tensor(rr6[:], rr6[:], rr2[:], ALU.mult)  # rr8
            den8 = wt("den8")
            nc.vector.tensor_tensor(den8[:], dr8[:], rr6[:], ALU.add)
            i8 = wt("i8")
            nc.vector.reciprocal_approx_fast(i8[:], den8[:])
            t8 = wt("t8")
            nc.vector.tensor_tensor(t8[:], qq[:], i8[:], ALU.mult)
            D = wt("D")
            nc.vector.scalar_tensor_tensor(D[:], i6[:], S6 / S8, t8[:], ALU.mult, ALU.add)
            Dwb = bt("Dwb")
            nc.scalar.mul(Dwb[:], D[:], DSCALE)

            # vjD[r] = wj[r] * Dw ; z[s] = sum_r M[5r+s] * vjD[r]
            vjD = bpool.tile([P, 5, W], BF16, tag="vjD")
            nc.vector.tensor_tensor(
                vjD[:], wj, Dwb[:].unsqueeze(1).to_broadcast([P, 5, W]), ALU.mult
            )
            # zf[5r+s] = M[5r+s] * vjD[r] in one broadcast mult, then fold r
            zf = bpool.tile([P, 25, W], BF16, tag="zf")
            nc.vector.tensor_tensor(
                zf[:].rearrange("p (r s) w -> p r s w", r=5),
                mb.rearrange("p (r s) w -> p r s w", r=5),
                vjD[:].unsqueeze(2).to_broadcast([P, 5, 5, W]), ALU.mult,
            )
            z = bpool.tile([P, 5, W], BF16, tag="z")
            nc.vector.tensor_tensor(z[:], zf[:, 0:5, :], zf[:, 5:10, :], ALU.add)
            nc.vector.tensor_tensor(z[:], z[:], zf[:, 10:15, :], ALU.add)
            nc.vector.tensor_tensor(z[:], z[:], zf[:, 15:20, :], ALU.add)
            nc.vector.tensor_tensor(z[:], z[:], zf[:, 20:25, :], ALU.add)

            Sp = w5pool.tile([P, 5, n_p], F32, tag="Sp")
            nc.vector.tensor_reduce(
                Sp[:], z[:].rearrange("p s (a l) -> p s a l", a=n_p), AX.X, ALU.add
            )
            junk = w5pool.tile([P, 5, n_p], F32, tag="junk")
            nc.vector.tensor_tensor(
                junk[:], Sp[:], att[:, 1:6, acol:acol + n_p], ALU.mult
            )
            ep = persist.tile([P, 1], F32, tag="ep", name="ep")
            nc.vector.tensor_reduce(ep[:], junk[:], AX.XY, ALU.add)
            if pi == 0:
                eacc = persist.tile([P, 1], F32, name="eacc", tag="eacc")
                ecols = [eacc]
                nc.vector.tensor_copy(eacc[:], ep[:])
            else:
                nc.vector.tensor_tensor(ecols[0][:], ecols[0][:], ep[:], ALU.add)

        ones = persist.tile([P, 1], F32)
        nc.vector.memset(ones[:], 1.0)
        ps = ppool.tile([1, 1], F32)
        nc.tensor.matmul(ps[:], ones[:], ecols[-1][:], start=True, stop=True)
        esb = persist.tile([1, 1], F32)
        nc.scalar.copy(esb[:], ps[:])
        nc.sync.dma_start(eto[:], esb[:])
    nc.compile()
    return nc


def kernel(dr_vec, ref_cn_table, ref_c6_table, r4r2_table, rcov_table, numbers, idx):
    # smooth_cutoff(dr, 20, 25) and (55, 60) are identically 1 for this data
    assert np.sqrt((dr_vec.astype(np.float64) ** 2).sum(-1)).max() / BOHR < 19.0
    prep = _prep_static(dr_vec, ref_cn_table, ref_c6_table, r4r2_table, rcov_table,
                        numbers, idx)

    key1 = ("l1", tuple(prep["pieces"]), prep["COLS"], prep["ACOLS"])
    if key1 not in _cache:
        _cache[key1] = _build_l1(prep["pieces"], prep["COLS"], prep["ACOLS"])
    nc1 = _cache[key1]

    in1 = [
        {"s1": c["s1"].reshape(P, -1), "at1": c["at1"].reshape(P, -1)}
        for c in prep["cores"]
    ]
    global LAST_R1, LAST_R2
    r1 = run_bass_kernel_spmd(nc1, in1, list(range(NCORES)), trace=TRACE)

    prep2 = _join(
        prep,
        [r1.results[k]["w5"] for k in range(NCORES)],
        [r1.results[k]["wsum"] for k in range(NCORES)],
    )

    key2 = ("l2", tuple(prep2["pieces"]), prep2["COLS"], prep2["ACOLS"])
    if key2 not in _cache:
        _cache[key2] = _build_l2(prep2["pieces"], prep2["COLS"], prep2["ACOLS"])
    nc2 = _cache[key2]

    in2 = [
        {"s2": c["s2"].reshape(P, -1), "at2": c["at2"].reshape(P, -1)}
        for c in prep2["cores"]
    ]
    r2 = run_bass_kernel_spmd(nc2, in2, list(range(NCORES)), trace=TRACE)
    LAST_R1, LAST_R2 = r1, r2

    parts = [r2.results[k]["etot"].reshape(()) for k in range(NCORES)]
    return np.float32(np.sum(np.stack(parts)))


# revision 14
# speedup vs baseline: 1.2131x; 1.2131x over previous
"""Trainium2 Bass kernel for the DispaxD3 two-body dispersion energy.

Strategy (8 NeuronCores, SPMD):
  - Edges are sorted by their i-atom and sharded across cores at atom
    boundaries.  Per-core edge slots are laid out in degree-bucketed padded
    runs [128 partitions, n_atom_cols, L] so the per-atom segment sum and
    broadcasts are regular strided vector ops.
  - Launch 1 (full 3.2M-edge stream, fp16): computes coordination numbers,
    then the normalized gaussian reference weights AND the raw weight sum
    per atom, all on device.
  - Pruning: the gaussian weight sum w_a = sum_r exp(-4 (ref_ar - cn_a)^2)
    underflows for all but ~8% of atoms (cn is large); an edge's C6 term is
    bounded by ~2.7e-3 * min(1, w_i/eps) * min(1, w_j/eps), so edges where
    either endpoint has w < 1e-12 contribute < 2e-6 relative in total.  The
    host keeps only edges with BOTH endpoints active (pure comparisons and
    gathers, no host arithmetic) -- ~0.7% of edges survive.
  - Launch 2 runs the exact bf16 5x5 C6 bilinear + BJ damping on the tiny
    kept-edge stream, segment-reduces per atom, dots with the i-atom
    weights and reduces to one scalar per core.  Host sums the 8 partials
    (the "all-reduce").
  - All element data is host-gathered into the streams; all floating point
    math happens on device.  Host does joins/selection/layout only.
"""

import sys

sys.path.insert(0, "/opt/trn_rl_repo")

from contextlib import ExitStack

import ml_dtypes
import numpy as np

import concourse.bacc as bacc
import concourse.bass as bass
import concourse.mybir as mybir
import concourse.tile as tile
from concourse.bass_utils import run_bass_kernel_spmd

F32 = mybir.dt.float32
F16 = mybir.dt.float16
BF16 = mybir.dt.bfloat16
AF = mybir.ActivationFunctionType
ALU = mybir.AluOpType
AX = mybir.AxisListType

BOHR = 0.5291772105638411
HA = 27.211386024367243
S6, S8, A1, A2 = 1.0, 0.7875, 0.4289, 4.4407
KCN = 16.0
WF = 4.0
EPS32 = float(np.finfo(np.float32).eps)
WSUM_CUT = 1e-12  # activity threshold on the raw gaussian weight sum

NCORES = 8
P = 128
LS1 = [12, 16, 20, 24, 28, 32, 36, 40, 44, 48, 56, 64, 96, 128, 256, 384]
LS2 = [16, 32, 64, 128, 256, 384]
MAXCOLS1 = 1536
MAXCOLS2 = 576
RCJ_PAD = -4.0  # pad-slot rcov_j: makes rc<=-2.5 so sigmoid count underflows to 0

SLOT1 = 4  # launch-1 planar fp16 lanes: dx dy dz rcov_j
NL2 = 34  # launch-2 planar bf16 lanes: dx dy dz r4r2_j wj[5] c6block[25]

_cache = {}
REPEAT = 1
TRACE = False
LAST_R1 = None
LAST_R2 = None


def _geometry(deg_lists, LS, maxcols):
    """Unified piece geometry from per-core degree lists (atoms to place)."""
    percore = []
    for degs in deg_lists:
        li = np.searchsorted(LS, degs, side="left")
        if len(degs):
            assert li.max() < len(LS), f"degree {degs.max()} exceeds bucket table"
        percore.append(np.bincount(li, minlength=len(LS)))
    nmax = np.stack(percore).max(axis=0)
    nmax = ((nmax + P - 1) // P) * P

    pieces = []  # (L, n_p, scol_off, acol_off)
    groups = []  # per bucket: (L, n_atoms, scol_off, acol_off)
    scol = 0
    acol = 0
    for bi, L in enumerate(LS):
        n = int(nmax[bi])
        groups.append((L, n, scol, acol))
        if n == 0:
            continue
        n_cols = n // P
        npp = max(1, maxcols // L)
        c = 0
        while c < n_cols:
            take = min(npp, n_cols - c)
            pieces.append((L, take, scol + c * L, acol + c))
            c += take
        scol += n_cols * L
        acol += n_cols
    return pieces, groups, scol, acol


def _place(li, atom_ids, groups, ACOLS):
    """Per-core placement of atoms (ascending ids) into the bucket grid."""
    n = len(atom_ids)
    part = np.empty(n, np.int64)
    scolb = np.empty(n, np.int64)
    agrid = np.full((P, ACOLS), -1, np.int64)
    for bi, (L, _, scol0, acol0) in enumerate(groups):
        sel = np.nonzero(li == bi)[0]
        if len(sel) == 0:
            continue
        t = np.arange(len(sel))
        c = t // P
        p = t % P
        part[sel] = p
        scolb[sel] = scol0 + c * L
        agrid[p, acol0 + c] = atom_ids[sel]
    return part, scolb, agrid


def _prep_static(dr_vec, ref_cn_table, ref_c6_table, r4r2_table, rcov_table, numbers, idx):
    N = numbers.shape[0]
    E = idx.shape[1]
    i = idx[0].astype(np.int64)
    j = idx[1].astype(np.int64)

    counts = np.bincount(i, minlength=N)
    ccum = np.concatenate([[0], np.cumsum(counts)])
    targets = [E * k // NCORES for k in range(1, NCORES)]
    cuts = [0] + [int(np.searchsorted(ccum, t)) for t in targets] + [N]
    atom_ranges = [(cuts[k], cuts[k + 1]) for k in range(NCORES)]

    deg1 = [counts[a0:a1] for a0, a1 in atom_ranges]
    pieces, groups, COLS, ACOLS = _geometry(deg1, LS1, MAXCOLS1)

    order = np.argsort(i, kind="stable")
    i_s = i[order]
    pos = np.arange(E, dtype=np.int64) - ccum[i_s]

    rcov_a = rcov_table[numbers]
    refcn_a = ref_cn_table[numbers]

    cores = []
    for k, (a0, a1) in enumerate(atom_ranges):
        degs = counts[a0:a1]
        li = np.searchsorted(LS1, degs, side="left")
        part, scolb, agrid = _place(li, np.arange(a0, a1), groups, ACOLS)

        e0, e1 = ccum[a0], ccum[a1]
        eo = order[e0:e1]
        il = i_s[e0:e1] - a0
        pp = part[il]
        cc = scolb[il] + pos[e0:e1]

        s1 = np.zeros((P, SLOT1, COLS), np.float16)
        s1[:, 3, :] = RCJ_PAD
        s1[pp, 0, cc] = dr_vec[eo, 0]
        s1[pp, 1, cc] = dr_vec[eo, 1]
        s1[pp, 2, cc] = dr_vec[eo, 2]
        s1[pp, 3, cc] = rcov_a[j[eo]]

        am = agrid >= 0
        at1 = np.zeros((P, 6, ACOLS), np.float32)
        at1[:, 0, :][am] = rcov_a[agrid[am]]
        tmp = np.zeros((P, ACOLS, 5), np.float32)
        tmp[am] = refcn_a[agrid[am]]
        at1[:, 1:6, :] = tmp.transpose(0, 2, 1)

        cores.append(dict(s1=s1, at1=at1, agrid=agrid, am=am, eo=eo, a0=a0, a1=a1))

    return dict(
        pieces=pieces, COLS=COLS, ACOLS=ACOLS, cores=cores, N=N, E=E,
        i=i, j=j, dr_vec=dr_vec, numbers=numbers,
        r4r2_table=r4r2_table, ref_c6_table=ref_c6_table,
        atom_ranges=atom_ranges,
    )


def _join(prep, w5_list, wsum_list):
    """Select contributing edges from launch-1's per-atom weight sums and
    build the compact launch-2 inputs.  Comparisons + gathers only."""
    N, ACOLS = prep["N"], prep["ACOLS"]
    i, j = prep["i"], prep["j"]
    numbers = prep["numbers"]
    dr_vec = prep["dr_vec"]
    r4r2_a = prep["r4r2_table"][numbers]

    ws_full = np.zeros(N, np.float32)
    w_full = np.zeros((N, 5), np.float32)
    grids = []
    for k, c in enumerate(prep["cores"]):
        m = c["am"]
        w5r = np.asarray(w5_list[k]).reshape(P, 5, ACOLS).transpose(0, 2, 1)
        grids.append(w5r)
        ws_full[c["agrid"][m]] = np.asarray(wsum_list[k]).reshape(P, ACOLS)[m]
        w_full[c["agrid"][m]] = w5r[m]

    active = ws_full > WSUM_CUT
    keep = active[i] & active[j]
    kcounts = np.bincount(i[keep], minlength=N)

    deg2 = []
    aid2 = []
    for a0, a1 in prep["atom_ranges"]:
        loc = kcounts[a0:a1]
        sel = np.nonzero(loc > 0)[0]
        deg2.append(loc[sel])
        aid2.append(sel + a0)

    pieces2, groups2, COLS2, ACOLS2 = _geometry(deg2, LS2, MAXCOLS2)

    r25 = np.arange(25)
    r5 = np.arange(5)
    cores2 = []
    for k, c in enumerate(prep["cores"]):
        li2 = np.searchsorted(LS2, deg2[k], side="left")
        part2, scolb2, agrid2 = _place(li2, aid2[k], groups2, ACOLS2)

        eo = c["eo"]
        ke = eo[keep[eo]]  # kept edges, still i-sorted
        ki = i[ke]
        fidx = np.searchsorted(aid2[k], ki)  # position of atom in aid2
        kpos = np.arange(len(ki)) - np.searchsorted(ki, ki)  # rank within run
        pp2 = part2[fidx]
        cc2 = scolb2[fidx] + kpos

        kj = j[ke]
        s2 = np.zeros((P, NL2, COLS2), ml_dtypes.bfloat16)
        s2[pp2, 0, cc2] = dr_vec[ke, 0]
        s2[pp2, 1, cc2] = dr_vec[ke, 1]
        s2[pp2, 2, cc2] = dr_vec[ke, 2]
        s2[pp2, 3, cc2] = r4r2_a[kj]
        s2[pp2[:, None], 4 + r5[None, :], cc2[:, None]] = w_full[kj]
        bfk = prep["ref_c6_table"][numbers[kj], numbers[ki]].reshape(-1, 25)
        s2[pp2[:, None], 9 + r25[None, :], cc2[:, None]] = bfk.astype(ml_dtypes.bfloat16)

        am2 = agrid2 >= 0
        at2 = np.zeros((P, 6, ACOLS2), np.float32)
        at2[:, 0, :][am2] = r4r2_a[agrid2[am2]]
        tmp = np.zeros((P, ACOLS2, 5), np.float32)
        tmp[am2] = w_full[agrid2[am2]]
        at2[:, 1:6, :] = tmp.transpose(0, 2, 1)

        cores2.append(dict(s2=s2, at2=at2))

    return dict(pieces=pieces2, COLS=COLS2, ACOLS=ACOLS2, cores=cores2)


def _new_nc():
    return bacc.Bacc("TRN2", target_bir_lowering=False, debug=False, num_devices=NCORES)


def _build_l1(pieces, COLS, ACOLS):
    nc = _new_nc()
    s1 = nc.declare_dram_parameter("s1", [P, SLOT1 * COLS], F16, isOutput=False)
    at1 = nc.declare_dram_parameter("at1", [P, 6 * ACOLS], F32, isOutput=False)
    w5o = nc.declare_dram_parameter("w5", [P, 5 * ACOLS], F32, isOutput=True)
    wso = nc.declare_dram_parameter("wsum", [P, ACOLS], F32, isOutput=True)
    s1v = s1[:].rearrange("p (m c) -> p m c", m=SLOT1)

    with ExitStack() as ctx:
        tc = ctx.enter_context(tile.TileContext(nc))
        persist = ctx.enter_context(tc.tile_pool(name="persist", bufs=1))
        spool = ctx.enter_context(tc.tile_pool(name="stream", bufs=3))
        wpool = ctx.enter_context(tc.tile_pool(name="work", bufs=3))

        cn_t = persist.tile([P, ACOLS], F32)
        at_t = persist.tile([P, 6, ACOLS], F32)
        nc.sync.dma_start(at_t[:], at1[:].rearrange("p (f a) -> p f a", f=6))
        b_tiny = persist.tile([P, 1], F32)
        # dr^2 floor 1e-4 Bohr^2: keeps rdr <= 100 so fp16 targ cannot
        # overflow; distorts only edges with dr < 0.01 Bohr (none expected)
        nc.vector.memset(b_tiny[:], 1e-4)
        b_negk = persist.tile([P, 1], F32)
        nc.vector.memset(b_negk[:], -KCN)
        targ_f = persist.tile([P, COLS], F16)
        cnt_f = persist.tile([P, COLS], F16)

        # Main loop keeps the ACT engine inside one activation-table set
        # (square+sqrt coexist); sigmoid runs as a second pass so the 1.3us
        # table load happens once per launch instead of twice per piece.
        for _rep in range(REPEAT):
          for (L, n_p, scol, acol) in pieces:
            W = n_p * L
            st = spool.tile([P, SLOT1, W], F16, tag="st")
            nc.sync.dma_start(st[:], s1v[:, :, scol:scol + W])

            # engine balance: x,y squares on DVE, z square on ACT,
            # |d|^2 adds on GPSIMD -- spreads ~11 element-passes 3 ways
            sq3 = wpool.tile([P, 3, W], F16, tag="sq3")
            nc.vector.tensor_tensor(
                sq3[:, 0:2, :], st[:, 0:2, :], st[:, 0:2, :], ALU.mult
            )
            nc.scalar.activation(sq3[:, 2, :], st[:, 2, :], AF.Square)
            s_ = wpool.tile([P, W], F16, tag="s_")
            nc.gpsimd.tensor_tensor(s_[:], sq3[:, 0, :], sq3[:, 1, :], ALU.add)
            nc.gpsimd.tensor_tensor(s_[:], s_[:], sq3[:, 2, :], ALU.add)
            dr = wpool.tile([P, W], F32, tag="dr")
            # dr = sqrt(|d|^2/BOHR^2 + tiny); tiny keeps pad slots finite
            nc.scalar.activation(dr[:], s_[:], AF.Sqrt, scale=1.0 / BOHR**2, bias=b_tiny[:])
            rdr = wpool.tile([P, W], F32, tag="rdr")
            nc.vector.reciprocal_approx_fast(rdr[:], dr[:])
            rc = wpool.tile([P, W], F16, tag="rc")
            rci = at_t[:, 0, acol:acol + n_p].unsqueeze(-1).to_broadcast([P, n_p, L])
            nc.vector.tensor_tensor(
                rc[:].rearrange("p (a l) -> p a l", a=n_p),
                st[:, 3, :].rearrange("p (a l) -> p a l", a=n_p), rci, ALU.add,
            )
            # pad slots: rc <= -2.5, rdr ~ 1e15 -> targ very negative -> count 0
            nc.vector.tensor_tensor(targ_f[:, scol:scol + W], rc[:], rdr[:], ALU.mult)

          for (L, n_p, scol, acol) in pieces:
            W = n_p * L
            nc.scalar.activation(
                cnt_f[:, scol:scol + W], targ_f[:, scol:scol + W],
                AF.Sigmoid, scale=KCN, bias=b_negk[:],
            )
            nc.vector.tensor_reduce(
                cn_t[:, acol:acol + n_p],
                cnt_f[:, scol:scol + W].rearrange("p (a l) -> p a l", a=n_p),
                AX.X, ALU.add,
            )

        # ---- per-atom gaussian weights from cn (tiny: 5*ACOLS elements)
        w5p = persist.tile([P, 5, ACOLS], F32)
        nc.vector.tensor_tensor(
            w5p[:], at_t[:, 1:6, :],
            cn_t[:].unsqueeze(1).to_broadcast([P, 5, ACOLS]), ALU.subtract,
        )
        nc.scalar.activation(w5p[:], w5p[:], AF.Square)
        nc.scalar.activation(w5p[:], w5p[:], AF.Exp, scale=-WF)
        wsum = persist.tile([P, ACOLS], F32)
        nc.vector.tensor_tensor(wsum[:], w5p[:, 0, :], w5p[:, 1, :], ALU.add)
        nc.vector.tensor_tensor(wsum[:], wsum[:], w5p[:, 2, :], ALU.add)
        nc.vector.tensor_tensor(wsum[:], wsum[:], w5p[:, 3, :], ALU.add)
        nc.vector.tensor_tensor(wsum[:], wsum[:], w5p[:, 4, :], ALU.add)
        nc.sync.dma_start(wso[:], wsum[:])
        wse = persist.tile([P, ACOLS], F32)
        nc.vector.tensor_scalar_add(wse[:], wsum[:], EPS32)
        winv = persist.tile([P, ACOLS], F32)
        nc.vector.reciprocal_approx_fast(winv[:], wse[:])
        nc.vector.tensor_tensor(
            w5p[:], w5p[:], winv[:].unsqueeze(1).to_broadcast([P, 5, ACOLS]), ALU.mult
        )
        nc.sync.dma_start(w5o[:].rearrange("p (f a) -> p f a", f=5), w5p[:])
    nc.compile()
    return nc


def _build_l2(pieces, COLS, ACOLS):
    nc = _new_nc()
    s2 = nc.declare_dram_parameter("s2", [P, NL2 * COLS], BF16, isOutput=False)
    at2 = nc.declare_dram_parameter("at2", [P, 6 * ACOLS], F32, isOutput=False)
    eto = nc.declare_dram_parameter("etot", [1, 1], F32, isOutput=True)
    s2v = s2[:].rearrange("p (m c) -> p m c", m=NL2)

    DSCALE = -0.5 * HA * S8  # energy scale folded into the damping factor

    with ExitStack() as ctx:
        tc = ctx.enter_context(tile.TileContext(nc))
        persist = ctx.enter_context(tc.tile_pool(name="persist", bufs=1))
        spool = ctx.enter_context(tc.tile_pool(name="stream", bufs=2))
        wpool = ctx.enter_context(tc.tile_pool(name="work", bufs=2))
        w5pool = ctx.enter_context(tc.tile_pool(name="work5", bufs=2))
        bpool = ctx.enter_context(tc.tile_pool(name="workb", bufs=2))
        ppool = ctx.enter_context(tc.tile_pool(name="psum", bufs=1, space="PSUM"))

        att = persist.tile([P, 6, ACOLS], F32)
        nc.sync.dma_start(att[:], at2[:].rearrange("p (f a) -> p f a", f=6))
        b_a2 = persist.tile([P, 1], F32)
        nc.vector.memset(b_a2[:], A2)
        r43 = persist.tile([P, ACOLS], F32)
        nc.vector.tensor_scalar_mul(r43[:], att[:, 0, :], 3.0)

        ecols = []
        for _rep in range(REPEAT):
          for pi, (L, n_p, scol, acol) in enumerate(pieces):
            W = n_p * L
            st = spool.tile([P, NL2, W], BF16, tag="st")
            nc.sync.dma_start(st[:], s2v[:, :, scol:scol + W])
            xyz = st[:, 0:3, :]
            r4 = st[:, 3, :]
            wj = st[:, 4:9, :]
            mb = st[:, 9:34, :]

            def wt(tag):
                return wpool.tile([P, W], F32, tag=tag, name=tag)

            def bt(tag):
                return bpool.tile([P, W], BF16, tag=tag, name=tag)

            sq3 = bpool.tile([P, 3, W], BF16, tag="sq3")
            nc.scalar.activation(sq3[:], xyz, AF.Square)
            s_ = bt("s_")
            nc.vector.tensor_tensor(s_[:], sq3[:, 0, :], sq3[:, 1, :], ALU.add)
            nc.vector.tensor_tensor(s_[:], s_[:], sq3[:, 2, :], ALU.add)
            t_ = bt("t_")
            nc.scalar.activation(t_[:], s_[:], AF.Square, scale=1.0 / BOHR**2)
            dr6 = bt("dr6")
            nc.vector.scalar_tensor_tensor(
                dr6[:], t_[:], 1.0 / BOHR**2, s_[:], ALU.mult, ALU.mult
            )
            dr8 = bt("dr8")
            nc.vector.scalar_tensor_tensor(
                dr8[:], dr6[:], 1.0 / BOHR**2, s_[:], ALU.mult, ALU.mult
            )
            qq = wt("qq")
            r4ib = r43[:, acol:acol + n_p].unsqueeze(-1).to_broadcast([P, n_p, L])
            nc.vector.tensor_tensor(
                qq[:].rearrange("p (a l) -> p a l", a=n_p),
                r4.rearrange("p (a l) -> p a l", a=n_p), r4ib, ALU.mult,
            )
            rrs = bt("rrs")
            nc.scalar.activation(rrs[:], qq[:], AF.Sqrt, scale=A1 * A1)
            rr2 = bt("rr2")
            nc.scalar.activation(rr2[:], rrs[:], AF.Square, bias=b_a2[:])
            t2_ = bt("rrs")
            nc.scalar.activation(t2_[:], rr2[:], AF.Square)
            rr6 = bt("rr6")
            nc.vector.tensor_tensor(rr6[:], t2_[:], rr2[:], ALU.mult)
            den6 = wt("den6")
            nc.vector.tensor_tensor(den6[:], dr6[:], rr6[:], ALU.add)
            i6 = wt("i6")
            nc.vector.reciprocal_approx_fast(i6[:], den6[:])
            nc.vector.tensor_tensor(rr6[:], rr6[:], rr2[:], ALU.mult)  # rr8
            den8 = wt("den8")
            nc.vector.tensor_tensor(den8[:], dr8[:], rr6[:], ALU.add)
            i8 = wt("i8")
            nc.vector.reciprocal_approx_fast(i8[:], den8[:])
            t8 = wt("t8")
            nc.vector.tensor_tensor(t8[:], qq[:], i8[:], ALU.mult)
            D = wt("D")
            nc.vector.scalar_tensor_tensor(D[:], i6[:], S6 / S8, t8[:], ALU.mult, ALU.add)
            Dwb = bt("Dwb")
            nc.scalar.mul(Dwb[:], D[:], DSCALE)

            # vjD[r] = wj[r] * Dw ; z[s] = sum_r M[5r+s] * vjD[r]
            vjD = bpool.tile([P, 5, W], BF16, tag="vjD")
            nc.vector.tensor_tensor(
                vjD[:], wj, Dwb[:].unsqueeze(1).to_broadcast([P, 5, W]), ALU.mult
            )
            # zf[5r+s] = M[5r+s] * vjD[r] in one broadcast mult, then fold r
            zf = bpool.tile([P, 25, W], BF16, tag="zf")
            nc.vector.tensor_tensor(
                zf[:].rearrange("p (r s) w -> p r s w", r=5),
                mb.rearrange("p (r s) w -> p r s w", r=5),
                vjD[:].unsqueeze(2).to_broadcast([P, 5, 5, W]), ALU.mult,
            )
            z = bpool.tile([P, 5, W], BF16, tag="z")
            nc.vector.tensor_tensor(z[:], zf[:, 0:5, :], zf[:, 5:10, :], ALU.add)
            nc.vector.tensor_tensor(z[:], z[:], zf[:, 10:15, :], ALU.add)
            nc.vector.tensor_tensor(z[:], z[:], zf[:, 15:20, :], ALU.add)
            nc.vector.tensor_tensor(z[:], z[:], zf[:, 20:25, :], ALU.add)

            Sp = w5pool.tile([P, 5, n_p], F32, tag="Sp")
            nc.vector.tensor_reduce(
                Sp[:], z[:].rearrange("p s (a l) -> p s a l", a=n_p), AX.X, ALU.add
            )
            junk = w5pool.tile([P, 5, n_p], F32, tag="junk")
            nc.vector.tensor_tensor(
                junk[:], Sp[:], att[:, 1:6, acol:acol + n_p], ALU.mult
            )
            ep = persist.tile([P, 1], F32, tag="ep", name="ep")
            nc.vector.tensor_reduce(ep[:], junk[:], AX.XY, ALU.add)
            if pi == 0:
                eacc = persist.tile([P, 1], F32, name="eacc", tag="eacc")
                ecols = [eacc]
                nc.vector.tensor_copy(eacc[:], ep[:])
            else:
                nc.vector.tensor_tensor(ecols[0][:], ecols[0][:], ep[:], ALU.add)

        ones = persist.tile([P, 1], F32)
        nc.vector.memset(ones[:], 1.0)
        ps = ppool.tile([1, 1], F32)
        nc.tensor.matmul(ps[:], ones[:], ecols[-1][:], start=True, stop=True)
        esb = persist.tile([1, 1], F32)
        nc.scalar.copy(esb[:], ps[:])
        nc.sync.dma_start(eto[:], esb[:])
    nc.compile()
    return nc


def kernel(dr_vec, ref_cn_table, ref_c6_table, r4r2_table, rcov_table, numbers, idx):
    # smooth_cutoff(dr, 20, 25) and (55, 60) are identically 1 for this data
    assert np.sqrt((dr_vec.astype(np.float64) ** 2).sum(-1)).max() / BOHR < 19.0
    prep = _prep_static(dr_vec, ref_cn_table, ref_c6_table, r4r2_table, rcov_table,
                        numbers, idx)

    key1 = ("l1", tuple(prep["pieces"]), prep["COLS"], prep["ACOLS"])
    if key1 not in _cache:
        _cache[key1] = _build_l1(prep["pieces"], prep["COLS"], prep["ACOLS"])
    nc1 = _cache[key1]

    in1 = [
        {"s1": c["s1"].reshape(P, -1), "at1": c["at1"].reshape(P, -1)}
        for c in prep["cores"]
    ]
    global LAST_R1, LAST_R2
    r1 = run_bass_kernel_spmd(nc1, in1, list(range(NCORES)), trace=TRACE)

    prep2 = _join(
        prep,
        [r1.results[k]["w5"] for k in range(NCORES)],
        [r1.results[k]["wsum"] for k in range(NCORES)],
    )

    key2 = ("l2", tuple(prep2["pieces"]), prep2["COLS"], prep2["ACOLS"])
    if key2 not in _cache:
        _cache[key2] = _build_l2(prep2["pieces"], prep2["COLS"], prep2["ACOLS"])
    nc2 = _cache[key2]

    in2 = [
        {"s2": c["s2"].reshape(P, -1), "at2": c["at2"].reshape(P, -1)}
        for c in prep2["cores"]
    ]
    r2 = run_bass_kernel_spmd(nc2, in2, list(range(NCORES)), trace=TRACE)
    LAST_R1, LAST_R2 = r1, r2

    parts = [r2.results[k]["etot"].reshape(()) for k in range(NCORES)]
    return np.float32(np.sum(np.stack(parts)))
